# revision 1
# baseline (speedup 1.0000x reference)
"""Local softmax attention (GNN message passing) on 8 Trainium2 NeuronCores.

Math (per batch b, node n):
  q/k/v = x @ W{q,k,v}.T + b{q,k,v}              [N, 128], 8 heads x d=16
  scores[n,k,h] = sum_d q[n,h,d] * k[nbr(n,k),h,d] / sqrt(d)
  attn = softmax over k (32 neighbors)
  out[n,h,d] = sum_k attn[n,k,h] * v[nbr(n,k),h,d]

Sharding: 8 cores, each owning a 2048-node range (both batches).

The dominant cost on TRN2 is the neighbor gather: SWDGE descriptor
generation costs ~8.6 ns per gathered row (Q7 software), so k|v for BOTH
batches are packed into one 1KB DRAM row per node and each gathered row
serves both batch instances -> 65536 rows per core.  dma_gather is
limited to ~1024 indices per call (the 128-deep SWDGE descriptor ring:
2048 idxs hangs the exec unit) so each 128-node tile's 4096-row gather is
split into 4 calls.

Phase 1 (projections): every core redundantly computes k|v (bf16) for
all nodes of both batches on the TensorEngine (x^T tiles stationary,
rank-1 e0-row matmul adds the bias) and writes the packed rows to a
private DRAM scratch; q (bf16) for its own nodes stays in SBUF.

Phase 2 (per 128-node tile, per batch): VectorE does q*kg (bf16 2x),
a 4-level pairwise-add tree over d for the scores (last levels fp32),
softmax denominator, attn*vg (bf16 2x, exp expanded over d by ScalarE so
both operands are dense), a 5-level pairwise tree over k, and the 1/Z
scale.  ScalarE does the exp (reading scores with a step-0 broadcast AP
so the output is already expanded over d).

SPMD: all 8 cores run the identical program; per-core variation is data
only (each core's x^T is permuted so its own 2048 nodes come first, and
gather indices are remapped into that row space).
"""

import os
import sys

sys.path.insert(0, "/opt/trn_rl_repo")

from contextlib import ExitStack

import numpy as np

import concourse.bacc as bacc
import concourse.bass as bass
import concourse.tile as tile
from concourse import mybir

HEADS = 8
P = 128
NCALL = 4          # gather calls per tile (1024 idxs each)


class Cfg:
    def __init__(self, N=16384, K=32, C=128, n_cores=8, B=2):
        self.N, self.K, self.C, self.n_cores, self.B = N, K, C, n_cores, B
        self.N_own = N // n_cores
        self.n_all_tiles = N // P
        self.n_own_tiles = self.N_own // P
        self.d = C // HEADS


def _ap(base, dims):
    return bass.AP(tensor=base.tensor, offset=base.offset,
                   ap=[base.ap[0]] + [list(x) for x in dims])


def _off(base, elems):
    return bass.AP(tensor=base.tensor, offset=base.offset + elems,
                   ap=base.ap)


def build_nc(cfg: Cfg):
    N, K, C, B = cfg.N, cfg.K, cfg.C, cfg.B
    H3 = 3 * C
    R = 2 * B * C              # packed row elems (k|v per batch): 512
    f32, bf16, i16 = mybir.dt.float32, mybir.dt.bfloat16, mybir.dt.int16
    T_all, T_own = cfg.n_all_tiles, cfg.n_own_tiles
    d = cfg.d
    ni_call = K * P // NCALL
    k_call = K // NCALL
    cols_call = ni_call // 16
    idx_cols = NCALL * cols_call

    nc = bacc.Bacc("TRN2", target_bir_lowering=False, debug=False)

    f32r = mybir.dt.float32r
    xt = nc.dram_tensor("xt", [C, B * N], f32r, kind="ExternalInput")
    wqkv = nc.dram_tensor("wqkv", [C, H3], f32r, kind="ExternalInput")
    erow = nc.dram_tensor("erow", [C, P], f32, kind="ExternalInput")
    brow = nc.dram_tensor("brow", [C, H3], f32, kind="ExternalInput")
    idxw = nc.dram_tensor("idxw", [P, T_own * idx_cols], i16,
                          kind="ExternalInput")
    outp = nc.dram_tensor("out", [B * cfg.N_own, C], f32,
                          kind="ExternalOutput")

    with tile.TileContext(nc) as tc, ExitStack() as ctx:
        const = ctx.enter_context(tc.tile_pool(name="const", bufs=1))
        xload = ctx.enter_context(tc.tile_pool(name="xload", bufs=4))
        psum = ctx.enter_context(tc.tile_pool(name="psum", bufs=4, space="PSUM"))
        kvsb = ctx.enter_context(tc.tile_pool(name="kvsb", bufs=4))
        dram = ctx.enter_context(tc.tile_pool(name="dram", bufs=1, space="DRAM"))
        gath = ctx.enter_context(tc.tile_pool(name="gath", bufs=2))
        work = ctx.enter_context(tc.tile_pool(name="work", bufs=2))
        small = ctx.enter_context(tc.tile_pool(name="small", bufs=2))

        # --- constants ---
        wqkv_sb = const.tile([C, H3], f32r)
        nc.sync.dma_start(out=wqkv_sb[:], in_=wqkv[:, :])
        del erow  # bias handled via bqrep/bvrep; erow input kept for ABI
        bvrep_sb = const.tile([P, C], f32)   # bv replicated across partitions
        b0 = brow[0:1, 2 * C : 3 * C]
        nc.gpsimd.dma_start(
            out=bvrep_sb[:],
            in_=bass.AP(tensor=b0.tensor, offset=b0.offset, ap=[[0, P], [1, C]]))
        bqrep_sb = const.tile([P, C], f32)   # bq replicated across partitions
        q0 = brow[0:1, 0:C]
        nc.gpsimd.dma_start(
            out=bqrep_sb[:],
            in_=bass.AP(tensor=q0.tensor, offset=q0.offset, ap=[[0, P], [1, C]]))
        idx_sb = const.tile([P, T_own * idx_cols], i16)
        nc.sync.dma_start(out=idx_sb[:], in_=idxw[:, :])
        q_all = const.tile([P, T_own * B * C], bf16)   # [t][b][hd]

        kv_dram = dram.tile([N, R], bf16)

        # --- phase 1: projections (both batches, all nodes) ---
        # bk cancels in the softmax (constant over the neighbor axis) and
        # bv is added exactly at the end (sum_k attn == 1), so k|v rows are
        # written WITHOUT bias and only q (own tiles) gets its bias via the
        # rank-1 e0-row matmul.
        for b in range(B):
            for t in range(T_all):
                xt_t = xload.tile([P, P], f32r)
                nc.sync.dma_start(out=xt_t[:],
                                  in_=xt[:, b * N + t * P : b * N + (t + 1) * P])
                pt = psum.tile([P, H3], f32)
                if t < T_own:
                    nc.tensor.matmul(out=pt[:], lhsT=xt_t[:],
                                     rhs=wqkv_sb[:],
                                     start=True, stop=True)
                    q_slot = (t * B + b) * C
                    nc.vector.tensor_tensor(
                        out=q_all[:, q_slot : q_slot + C], in0=pt[:, 0:C],
                        in1=bqrep_sb[:], op=mybir.AluOpType.add)
                else:
                    nc.tensor.matmul(out=pt[:, C:H3],
                                     lhsT=xt_t[:],
                                     rhs=wqkv_sb[:, C:H3],
                                     start=True, stop=True)
                kv_t = kvsb.tile([P, 2 * C], bf16)
                nc.scalar.activation(out=kv_t[:], in_=pt[:, C:H3],
                                     func=mybir.ActivationFunctionType.Copy)
                nc.sync.dma_start(
                    out=kv_dram[t * P : (t + 1) * P, 2 * b * C : 2 * (b + 1) * C],
                    in_=kv_t[:])

        # --- phase 2: gather + attention ---
        for t in range(T_own):
            kvg = gath.tile([P, K, R], bf16)
            for i in range(NCALL):
                c0 = t * idx_cols + i * cols_call
                nc.gpsimd.dma_gather(
                    out_ap=kvg[:, i * k_call : (i + 1) * k_call, :],
                    in_ap=kv_dram[:],
                    idxs_ap=idx_sb[:, c0 : c0 + cols_call],
                    num_idxs=ni_call,
                    num_idxs_reg=ni_call,
                    elem_size=R,
                )

            for b in range(B):
                kg = kvg[:, :, 2 * b * C : 2 * b * C + C]        # (k, hd)
                vg = kvg[:, :, 2 * b * C + C : 2 * (b + 1) * C]
                qt = q_all[:, (t * B + b) * C : (t * B + b + 1) * C]

                # prod[(k,h,d)] = kg * q   (bf16 2x)
                prod = work.tile([P, K * C], bf16)
                nc.vector.tensor_tensor(
                    out=_ap(prod[:], [[C, K], [1, C]]),
                    in0=kg, in1=_ap(qt, [[0, K], [1, C]]),
                    op=mybir.AluOpType.mult)

                # scores = sum_d prod : 4-level pairwise tree over d
                # L1: (k,h,8) bf16, L2: (k,h,4) bf16, L3: (k,h,2) f32,
                # L4: (k,h) f32
                st1 = small.tile([P, K * HEADS * 8], bf16)
                nc.vector.tensor_tensor(
                    out=_ap(st1[:], [[8, K * HEADS], [1, 8]]),
                    in0=_ap(prod[:], [[d, K * HEADS], [1, 8]]),
                    in1=_ap(_off(prod[:], 8), [[d, K * HEADS], [1, 8]]),
                    op=mybir.AluOpType.add)
                st2 = small.tile([P, K * HEADS * 4], bf16)
                nc.vector.tensor_tensor(
                    out=_ap(st2[:], [[4, K * HEADS], [1, 4]]),
                    in0=_ap(st1[:], [[8, K * HEADS], [1, 4]]),
                    in1=_ap(_off(st1[:], 4), [[8, K * HEADS], [1, 4]]),
                    op=mybir.AluOpType.add)
                st3 = small.tile([P, K * HEADS * 2], f32)
                nc.vector.tensor_tensor(
                    out=_ap(st3[:], [[2, K * HEADS], [1, 2]]),
                    in0=_ap(st2[:], [[4, K * HEADS], [1, 2]]),
                    in1=_ap(_off(st2[:], 2), [[4, K * HEADS], [1, 2]]),
                    op=mybir.AluOpType.add)
                scores = small.tile([P, K * HEADS], f32)       # (k, h)
                nc.vector.tensor_tensor(
                    out=_ap(scores[:], [[1, K * HEADS]]),
                    in0=_ap(st3[:], [[2, K * HEADS]]),
                    in1=_ap(_off(st3[:], 1), [[2, K * HEADS]]),
                    op=mybir.AluOpType.add)

                # expx[(k,h,d)] = exp(scores/4) expanded over d (ScalarE)
                expx = work.tile([P, K * C], bf16)
                nc.scalar.activation(
                    out=_ap(expx[:], [[C, K], [d, HEADS], [1, d]]),
                    in_=_ap(scores[:], [[HEADS, K], [1, HEADS], [0, d]]),
                    func=mybir.ActivationFunctionType.Exp,
                    scale=1.0 / float(np.sqrt(d)))

                # z[h] = sum_k expx[k,h,0]
                z = small.tile([P, HEADS], f32)
                nc.vector.tensor_reduce(
                    out=z[:],
                    in_=_ap(expx[:], [[d, HEADS], [C, K]]),
                    axis=mybir.AxisListType.X, op=mybir.AluOpType.add)
                rz = small.tile([P, HEADS], f32)
                nc.vector.reciprocal(rz[:], z[:])

                # prod2 = expx * vg  (bf16 2x, both dense)
                prod2 = work.tile([P, K * C], bf16)
                nc.vector.tensor_tensor(
                    out=_ap(prod2[:], [[C, K], [1, C]]),
                    in0=vg, in1=_ap(expx[:], [[C, K], [1, C]]),
                    op=mybir.AluOpType.mult)

                # acc = sum_k prod2 : 5-level pairwise tree over k
                at1 = small.tile([P, K * C // 2], bf16)
                nc.vector.tensor_tensor(
                    out=at1[:], in0=prod2[:, 0 : K * C // 2],
                    in1=prod2[:, K * C // 2 : K * C],
                    op=mybir.AluOpType.add)
                at2 = small.tile([P, K * C // 4], bf16)
                nc.vector.tensor_tensor(
                    out=at2[:], in0=at1[:, 0 : K * C // 4],
                    in1=at1[:, K * C // 4 : K * C // 2],
                    op=mybir.AluOpType.add)
                at3 = small.tile([P, K * C // 8], bf16)
                nc.vector.tensor_tensor(
                    out=at3[:], in0=at2[:, 0 : K * C // 8],
                    in1=at2[:, K * C // 8 : K * C // 4],
                    op=mybir.AluOpType.add)
                at4 = small.tile([P, 2 * C], bf16)
                nc.vector.tensor_tensor(
                    out=at4[:], in0=at3[:, 0 : 2 * C], in1=at3[:, 2 * C : 4 * C],
                    op=mybir.AluOpType.add)
                acc = small.tile([P, C], f32)
                nc.vector.tensor_tensor(
                    out=acc[:], in0=at4[:, 0:C], in1=at4[:, C : 2 * C],
                    op=mybir.AluOpType.add)

                # out = acc * (1/z) + bv   (bv exact: sum_k attn == 1)
                sc = small.tile([P, C], f32)
                nc.vector.tensor_tensor(
                    out=sc[:], in0=acc[:],
                    in1=_ap(rz[:], [[1, HEADS], [0, d]]),
                    op=mybir.AluOpType.mult)
                outt = small.tile([P, C], f32)
                nc.vector.tensor_tensor(
                    out=outt[:], in0=sc[:], in1=bvrep_sb[:],
                    op=mybir.AluOpType.add)
                nc.sync.dma_start(
                    out=outp[b * cfg.N_own + t * P : b * cfg.N_own + (t + 1) * P, :],
                    in_=outt[:])

    nc.compile()
    return nc


def make_in_maps(cfg: Cfg, x, Wq, bq, Wk, bk, Wv, bv, neighbor_index):
    N, K, C, B = cfg.N, cfg.K, cfg.C, cfg.B
    T_own, N_own = cfg.n_own_tiles, cfg.N_own

    x = np.asarray(x, np.float32)
    wqkv = np.ascontiguousarray(np.concatenate(
        [np.asarray(Wq, np.float32).T, np.asarray(Wk, np.float32).T,
         np.asarray(Wv, np.float32).T], axis=1))
    erow = np.zeros((C, P), np.float32)
    erow[0, :] = 1.0
    brow = np.zeros((C, 3 * C), np.float32)
    brow[0, :] = np.concatenate(
        [np.asarray(bq, np.float32), np.asarray(bk, np.float32),
         np.asarray(bv, np.float32)])
    nbr = np.asarray(neighbor_index, np.int64)
    xtb = np.ascontiguousarray(x.transpose(0, 2, 1))   # [B, C, N]

    in_maps = []
    for c in range(cfg.n_cores):
        own = np.arange(c * N_own, (c + 1) * N_own)
        rest = np.concatenate(
            [np.arange(0, c * N_own), np.arange((c + 1) * N_own, N)])
        perm = np.concatenate([own, rest])
        inv = np.empty(N, np.int64)
        inv[perm] = np.arange(N)

        xt_c = np.ascontiguousarray(
            xtb[:, :, perm].transpose(1, 0, 2).reshape(C, B * N))

        nb = inv[nbr[own]]                                  # [N_own, K]
        vals = nb.reshape(T_own, P, K).transpose(0, 2, 1)   # [T, k, nl]
        vals = vals.reshape(T_own, NCALL, (K // NCALL) * P)
        a = vals.reshape(T_own, NCALL, (K // NCALL) * P // 16, 16)
        a = a.transpose(3, 0, 1, 2)                          # [16, T, NCALL, S]
        rep = np.tile(a, (8, 1, 1, 1))                       # [128, ...]
        idxw = np.ascontiguousarray(
            rep.reshape(P, T_own * (K * P // 16)).astype(np.int16))

        in_maps.append({
            "xt": xt_c, "wqkv": wqkv, "erow": erow, "brow": brow,
            "idxw": idxw,
        })
    return in_maps


_CACHE = {}


def _get_nc(cfg: Cfg):
    key = (cfg.N, cfg.K, cfg.C, cfg.n_cores, cfg.B)
    if key not in _CACHE:
        _CACHE[key] = build_nc(cfg)
    return _CACHE[key]


def kernel(x, Wq, bq, Wk, bk, Wv, bv, neighbor_index, _trace=False):
    from concourse.bass_utils import run_bass_kernel_spmd

    x = np.asarray(x)
    B, N, C = x.shape
    K = np.asarray(neighbor_index).shape[1]
    cfg = Cfg(N=N, K=K, C=C, n_cores=8, B=B)
    nc = _get_nc(cfg)
    in_maps = make_in_maps(cfg, x, Wq, bq, Wk, bk, Wv, bv, neighbor_index)
    res = run_bass_kernel_spmd(nc, in_maps, core_ids=list(range(cfg.n_cores)),
                               trace=_trace)
    out = np.empty((B, N, C), np.float32)
    for c in range(cfg.n_cores):
        o = res.results[c]["out"].reshape(B, cfg.N_own, C)
        out[:, c * cfg.N_own : (c + 1) * cfg.N_own, :] = o
    if _trace:
        kernel.last_results = res
    return out



# revision 4
# speedup vs baseline: 1.8794x; 1.8794x over previous
"""Local softmax attention (GNN message passing) on 8 Trainium2 NeuronCores.

Math (per batch b, node n):
  q/k/v = x @ W{q,k,v}.T + b{q,k,v}              [N, 128], 8 heads x d=16
  scores[n,k,h] = sum_d q[n,h,d] * k[nbr(n,k),h,d] / sqrt(d)
  attn = softmax over k (32 neighbors)
  out[n,h,d] = sum_k attn[n,k,h] * v[nbr(n,k),h,d]

Sharding: 8 cores, each owning a 2048-node range (both batches).

v2 changes vs baseline (1195us):
 - phase 1 in bf16 (x and W shipped as bf16): 4x faster matmuls, half the
   x DMA bytes.  Projections grouped 8 node-tiles per iteration: one
   256KB xt load, 8 matmuls accumulating into a 4-bank PSUM tile, one
   ACT copy, one batched kv store (512 DMAs -> ~72; the Sync engine's
   650ns/DMA issue serialized phase 1 at 400us).
 - the 4 dma_gather calls per tile round-robin over SWDGE queues 0-3:
   descriptor generation (8.9us per 1024-row call, one Q7 pair per
   queue) runs 4-wide instead of serialized (was 568us on queue 0).
 - phase 2 DVE ops merged across the two batches (one instruction over
   [k, b, hd]), 4-level score tree replaced by one tensor_reduce over d,
   and the per-tile chain software-pipelined so the ScalarE exp of tile
   t overlaps the DVE prod/scores of tile t+1.

kv row layout in DRAM (1KB, serves both batches per gathered row):
  [k_b0 (128 bf16) | v_b0 | k_b1 | v_b1]
"""

import os
import sys

sys.path.insert(0, "/opt/trn_rl_repo")

from contextlib import ExitStack

import numpy as np

import concourse.bacc as bacc
import concourse.bass as bass
import concourse.tile as tile
from concourse import mybir

HEADS = 8
P = 128
NCALL = 4          # gather calls per tile (1024 idxs each)
GT = 8             # node-tiles per phase-1 group


class Cfg:
    def __init__(self, N=16384, K=32, C=128, n_cores=8, B=2):
        self.N, self.K, self.C, self.n_cores, self.B = N, K, C, n_cores, B
        self.N_own = N // n_cores
        self.n_all_tiles = N // P
        self.n_own_tiles = self.N_own // P
        self.d = C // HEADS


def _ap(base, dims):
    return bass.AP(tensor=base.tensor, offset=base.offset,
                   ap=[base.ap[0]] + [list(x) for x in dims])


def _off(base, elems):
    return bass.AP(tensor=base.tensor, offset=base.offset + elems,
                   ap=base.ap)


def build_nc(cfg: Cfg):
    N, K, C, B = cfg.N, cfg.K, cfg.C, cfg.B
    H3 = 3 * C
    R = 2 * B * C              # packed row elems (k|v per batch): 512
    d = cfg.d
    f32, bf16, i16 = mybir.dt.float32, mybir.dt.bfloat16, mybir.dt.int16
    T_all, T_own = cfg.n_all_tiles, cfg.n_own_tiles
    NG = T_all // GT           # phase-1 groups per batch (16)
    ni_call = K * P // NCALL   # 1024
    k_call = K // NCALL        # 8
    cols_call = ni_call // 16  # 64
    idx_cols = NCALL * cols_call
    M = K * B * C              # merged free size (8192)

    nc = bacc.Bacc("TRN2", target_bir_lowering=False, debug=False,
                   num_swdge_queues=4)

    xt = nc.dram_tensor("xt", [C, B * N], bf16, kind="ExternalInput")
    wqkv = nc.dram_tensor("wqkv", [C, H3], bf16, kind="ExternalInput")
    brow = nc.dram_tensor("brow", [C, H3], f32, kind="ExternalInput")
    idxw = nc.dram_tensor("idxw", [P, T_own * idx_cols], i16,
                          kind="ExternalInput")
    outp = nc.dram_tensor("out", [B * cfg.N_own, C], f32,
                          kind="ExternalOutput")

    with tile.TileContext(nc) as tc, ExitStack() as ctx:
        const = ctx.enter_context(tc.tile_pool(name="const", bufs=1))
        xload = ctx.enter_context(tc.tile_pool(name="xload", bufs=3))
        psum = ctx.enter_context(tc.tile_pool(name="psum", bufs=2, space="PSUM"))
        kvsb = ctx.enter_context(tc.tile_pool(name="kvsb", bufs=2))
        dram = ctx.enter_context(tc.tile_pool(name="dram", bufs=1, space="DRAM"))
        gath = ctx.enter_context(tc.tile_pool(name="gath", bufs=2))
        work = ctx.enter_context(tc.tile_pool(name="work", bufs=1))
        small = ctx.enter_context(tc.tile_pool(name="small", bufs=1))

        # --- constants ---
        wqkv_sb = const.tile([C, H3], bf16)
        nc.sync.dma_start(out=wqkv_sb[:], in_=wqkv[:, :])
        bqrep_sb = const.tile([P, C], f32)   # bq replicated across partitions
        q0 = brow[0:1, 0:C]
        nc.gpsimd.dma_start(
            out=bqrep_sb[:],
            in_=bass.AP(tensor=q0.tensor, offset=q0.offset, ap=[[0, P], [1, C]]))
        bvrep2_sb = const.tile([P, 2 * C], f32)  # bv twice: [(b, hd)]
        b0 = brow[0:1, 2 * C : 3 * C]
        nc.gpsimd.dma_start(
            out=bvrep2_sb[:],
            in_=bass.AP(tensor=b0.tensor, offset=b0.offset,
                        ap=[[0, P], [0, 2], [1, C]]))
        idx_sb = const.tile([P, T_own * idx_cols], i16)
        nc.sync.dma_start(out=idx_sb[:], in_=idxw[:, :])
        q_all = const.tile([P, T_own * B * C], bf16)   # [t][b][hd]

        kv_dram = dram.tile([N, R], bf16)

        # --- phase 1: k|v projections, all nodes, both batches ---
        # bk cancels in the softmax (constant over the neighbor axis) and
        # bv is added exactly at the end (sum_k attn == 1), so k|v rows are
        # written WITHOUT bias; only q gets its bias (DVE add below).
        for b in range(B):
            for g in range(NG):
                n0 = b * N + g * GT * P
                xt_g = xload.tile([C, GT * P], bf16, tag="xt")
                nc.sync.dma_start(out=xt_g[:], in_=xt[:, n0 : n0 + GT * P])
                pt = psum.tile([P, GT * 2 * C], f32, tag="pp")   # 4 banks
                for c in range(GT):
                    nc.tensor.matmul(out=pt[:, c * 2 * C : (c + 1) * 2 * C],
                                     lhsT=xt_g[:, c * P : (c + 1) * P],
                                     rhs=wqkv_sb[:, C:H3],
                                     start=True, stop=True)
                kv_t = kvsb.tile([P, GT * 2 * C], bf16, tag="kv")
                nc.scalar.activation(out=kv_t[:], in_=pt[:],
                                     func=mybir.ActivationFunctionType.Copy)
                # rows g*GT*P .. +GT*P, cols [2bC, 2bC+256) of kv_dram
                kvd = kv_dram[:]
                nc.sync.dma_start(
                    out=bass.AP(tensor=kvd.tensor,
                                offset=kvd.offset + g * GT * P * R + 2 * b * C,
                                ap=[[R, P], [P * R, GT], [1, 2 * C]]),
                    in_=kv_t[:])

        # q projections for own tiles (first T_own tiles of each batch)
        for b in range(B):
            for g in range(T_own // GT):        # 2 groups of 8 tiles
                n0 = b * N + g * GT * P
                xt_g = xload.tile([C, GT * P], bf16, tag="xt")
                nc.sync.dma_start(out=xt_g[:], in_=xt[:, n0 : n0 + GT * P])
                pq = psum.tile([P, GT * 2 * C], f32, tag="pp")
                for c in range(GT):
                    nc.tensor.matmul(out=pq[:, c * C : (c + 1) * C],
                                     lhsT=xt_g[:, c * P : (c + 1) * P],
                                     rhs=wqkv_sb[:, 0:C],
                                     start=True, stop=True)
                qa = q_all[:]
                nc.vector.tensor_tensor(
                    out=bass.AP(tensor=qa.tensor,
                                offset=qa.offset + (g * GT * B + b) * C,
                                ap=[qa.ap[0], [B * C, GT], [1, C]]),
                    in0=pq[:, 0 : GT * C],
                    in1=_ap(bqrep_sb[:], [[0, GT], [1, C]]),
                    op=mybir.AluOpType.add)

        # --- phase 2: gather + attention, software-pipelined ---
        # iter t: gather(t), prod(t), scores(t), exp(t) [ACT]
        #         then finish tile t-1: z, rz, prodv, tree, scale, store.
        prev = None
        for t in range(T_own + 1):
            if t < T_own:
                kvg = gath.tile([P, K, R], bf16, tag="kvg")
                for i in range(NCALL):
                    c0 = t * idx_cols + i * cols_call
                    nc.gpsimd.dma_gather(
                        out_ap=kvg[:, i * k_call : (i + 1) * k_call, :],
                        in_ap=kv_dram[:],
                        idxs_ap=idx_sb[:, c0 : c0 + cols_call],
                        num_idxs=ni_call,
                        num_idxs_reg=ni_call,
                        elem_size=R,
                        queue_num=i,
                    )

                qt = q_all[:, t * B * C : (t + 1) * B * C]     # [(b, hd)]

                # prod[(k,b,hd)] = kg * q   (bf16 2x)
                prod = work.tile([P, M], bf16, tag="prod")
                nc.vector.tensor_tensor(
                    out=_ap(prod[:], [[B * C, K], [C, B], [1, C]]),
                    in0=_ap(kvg[:, 0, 0:C], [[R, K], [2 * C, B], [1, C]]),
                    in1=_ap(qt, [[0, K], [C, B], [1, C]]),
                    op=mybir.AluOpType.mult)

                # scores[(k,b,h)] = sum_d prod  (grouped reduce over d)
                scores = small.tile([P, K * B * HEADS], f32, tag="scores",
                                    bufs=2)
                nc.vector.tensor_reduce(
                    out=_ap(scores[:], [[1, K * B * HEADS]]),
                    in_=_ap(prod[:], [[d, K * B * HEADS], [1, d]]),
                    axis=mybir.AxisListType.X, op=mybir.AluOpType.add)

                # expx[(k,b,hd)] = exp(scores/4) expanded over d (ScalarE)
                expx = work.tile([P, M], bf16, tag="expx", bufs=2)
                for b in range(B):
                    nc.scalar.activation(
                        out=_ap(_off(expx[:], b * C),
                                [[B * C, K], [d, HEADS], [1, d]]),
                        in_=_ap(_off(scores[:], b * HEADS),
                                [[B * HEADS, K], [1, HEADS], [0, d]]),
                        func=mybir.ActivationFunctionType.Exp,
                        scale=1.0 / float(np.sqrt(d)))
            else:
                kvg = expx = None

            if prev is not None:
                pkvg, pexpx, pt_idx = prev

                # z[(b,h)] = sum_k expx[k,b,h,0]
                z = small.tile([P, B * HEADS], f32, tag="z")
                nc.vector.tensor_reduce(
                    out=z[:],
                    in_=_ap(pexpx[:], [[C, B], [d, HEADS], [B * C, K]]),
                    axis=mybir.AxisListType.X, op=mybir.AluOpType.add)
                rz = small.tile([P, B * HEADS], f32, tag="rz")
                nc.vector.reciprocal(rz[:], z[:])

                # prodv = expx * vg  (both dense, bf16 2x)
                prodv = work.tile([P, M], bf16, tag="prodv")
                nc.vector.tensor_tensor(
                    out=_ap(prodv[:], [[B * C, K], [C, B], [1, C]]),
                    in0=_ap(pkvg[:, 0, C : 2 * C], [[R, K], [2 * C, B], [1, C]]),
                    in1=_ap(pexpx[:], [[B * C, K], [C, B], [1, C]]),
                    op=mybir.AluOpType.mult)

                # acc[(b,hd)] = sum_k prodv : 5-level pairwise tree over k
                at1 = work.tile([P, M // 2], bf16, tag="at1")
                nc.vector.tensor_tensor(
                    out=at1[:], in0=prodv[:, 0 : M // 2],
                    in1=prodv[:, M // 2 : M], op=mybir.AluOpType.add)
                at2 = small.tile([P, M // 4], bf16, tag="at2")
                nc.vector.tensor_tensor(
                    out=at2[:], in0=at1[:, 0 : M // 4],
                    in1=at1[:, M // 4 : M // 2], op=mybir.AluOpType.add)
                at3 = small.tile([P, M // 8], bf16, tag="at3")
                nc.vector.tensor_tensor(
                    out=at3[:], in0=at2[:, 0 : M // 8],
                    in1=at2[:, M // 8 : M // 4], op=mybir.AluOpType.add)
                at4 = small.tile([P, M // 16], bf16, tag="at4")
                nc.vector.tensor_tensor(
                    out=at4[:], in0=at3[:, 0 : M // 16],
                    in1=at3[:, M // 16 : M // 8], op=mybir.AluOpType.add)
                acc = small.tile([P, B * C], f32, tag="acc")
                nc.vector.tensor_tensor(
                    out=acc[:], in0=at4[:, 0 : B * C],
                    in1=at4[:, B * C : 2 * B * C], op=mybir.AluOpType.add)

                # out = acc * (1/z) + bv   (bv exact: sum_k attn == 1)
                sc = small.tile([P, B * C], f32, tag="sc")
                nc.vector.tensor_tensor(
                    out=_ap(sc[:], [[C, B], [d, HEADS], [1, d]]),
                    in0=_ap(acc[:], [[C, B], [d, HEADS], [1, d]]),
                    in1=_ap(rz[:], [[HEADS, B], [1, HEADS], [0, d]]),
                    op=mybir.AluOpType.mult)
                outt = small.tile([P, B * C], f32, tag="outt", bufs=2)
                nc.vector.tensor_tensor(
                    out=outt[:], in0=sc[:], in1=bvrep2_sb[:],
                    op=mybir.AluOpType.add)
                op_ = outp[:, :]
                nc.sync.dma_start(
                    out=bass.AP(tensor=op_.tensor,
                                offset=op_.offset + pt_idx * P * C,
                                ap=[[C, P], [cfg.N_own * C, B], [1, C]]),
                    in_=outt[:])

            prev = (kvg, expx, t) if t < T_own else None

    nc.compile()
    return nc


def make_in_maps(cfg: Cfg, x, Wq, bq, Wk, bk, Wv, bv, neighbor_index):
    N, K, C, B = cfg.N, cfg.K, cfg.C, cfg.B
    T_own, N_own = cfg.n_own_tiles, cfg.N_own

    x = np.asarray(x, np.float32)
    wqkv = np.ascontiguousarray(np.concatenate(
        [np.asarray(Wq, np.float32).T, np.asarray(Wk, np.float32).T,
         np.asarray(Wv, np.float32).T], axis=1))
    wqkv_bf = _to_bf16(wqkv)
    brow = np.zeros((C, 3 * C), np.float32)
    brow[0, :] = np.concatenate(
        [np.asarray(bq, np.float32), np.asarray(bk, np.float32),
         np.asarray(bv, np.float32)])
    nbr = np.asarray(neighbor_index, np.int64)
    xtb = np.ascontiguousarray(x.transpose(0, 2, 1))   # [B, C, N]

    in_maps = []
    for c in range(cfg.n_cores):
        own = np.arange(c * N_own, (c + 1) * N_own)
        rest = np.concatenate(
            [np.arange(0, c * N_own), np.arange((c + 1) * N_own, N)])
        perm = np.concatenate([own, rest])
        inv = np.empty(N, np.int64)
        inv[perm] = np.arange(N)

        xt_c = np.ascontiguousarray(
            xtb[:, :, perm].transpose(1, 0, 2).reshape(C, B * N))

        nb = inv[nbr[own]]                                  # [N_own, K]
        vals = nb.reshape(T_own, P, K).transpose(0, 2, 1)   # [T, k, nl]
        vals = vals.reshape(T_own, NCALL, (K // NCALL) * P)
        a = vals.reshape(T_own, NCALL, (K // NCALL) * P // 16, 16)
        a = a.transpose(3, 0, 1, 2)                          # [16, T, NCALL, S]
        rep = np.tile(a, (8, 1, 1, 1))                       # [128, ...]
        idxw = np.ascontiguousarray(
            rep.reshape(P, T_own * (K * P // 16)).astype(np.int16))

        in_maps.append({
            "xt": _to_bf16(xt_c), "wqkv": wqkv_bf, "brow": brow,
            "idxw": idxw,
        })
    return in_maps


def _to_bf16(a: np.ndarray) -> np.ndarray:
    import ml_dtypes
    return np.ascontiguousarray(a.astype(ml_dtypes.bfloat16))


_CACHE = {}


def _get_nc(cfg: Cfg):
    key = (cfg.N, cfg.K, cfg.C, cfg.n_cores, cfg.B)
    if key not in _CACHE:
        _CACHE[key] = build_nc(cfg)
    return _CACHE[key]


def kernel(x, Wq, bq, Wk, bk, Wv, bv, neighbor_index, _trace=False):
    from concourse.bass_utils import run_bass_kernel_spmd

    x = np.asarray(x)
    B, N, C = x.shape
    K = np.asarray(neighbor_index).shape[1]
    cfg = Cfg(N=N, K=K, C=C, n_cores=8, B=B)
    nc = _get_nc(cfg)
    in_maps = make_in_maps(cfg, x, Wq, bq, Wk, bk, Wv, bv, neighbor_index)
    res = run_bass_kernel_spmd(nc, in_maps, core_ids=list(range(cfg.n_cores)),
                               trace=_trace)
    out = np.empty((B, N, C), np.float32)
    for c in range(cfg.n_cores):
        o = res.results[c]["out"].reshape(B, cfg.N_own, C)
        out[:, c * cfg.N_own : (c + 1) * cfg.N_own, :] = o
    if _trace:
        kernel.last_results = res
    return out


# revision 8
# speedup vs baseline: 1.9178x; 1.0204x over previous
"""Local softmax attention (GNN message passing) on 8 Trainium2 NeuronCores.

Math (per batch b, node n):
  q/k/v = x @ W{q,k,v}.T + b{q,k,v}              [N, 128], 8 heads x d=16
  scores[n,k,h] = sum_d q[n,h,d] * k[nbr(n,k),h,d] / sqrt(d)
  attn = softmax over k (32 neighbors)
  out[n,h,d] = sum_k attn[n,k,h] * v[nbr(n,k),h,d]

Sharding: 8 cores, each owning a 2048-node range (both batches).

v2 changes vs baseline (1195us):
 - phase 1 in bf16 (x and W shipped as bf16): 4x faster matmuls, half the
   x DMA bytes.  Projections grouped 8 node-tiles per iteration: one
   256KB xt load, 8 matmuls accumulating into a 4-bank PSUM tile, one
   ACT copy, one batched kv store (512 DMAs -> ~72; the Sync engine's
   650ns/DMA issue serialized phase 1 at 400us).
 - the 4 dma_gather calls per tile round-robin over SWDGE queues 0-3:
   descriptor generation (8.9us per 1024-row call, one Q7 pair per
   queue) runs 4-wide instead of serialized (was 568us on queue 0).
 - phase 2 DVE ops merged across the two batches (one instruction over
   [k, b, hd]), 4-level score tree replaced by one tensor_reduce over d,
   and the per-tile chain software-pipelined so the ScalarE exp of tile
   t overlaps the DVE prod/scores of tile t+1.

kv row layout in DRAM (1KB, serves both batches per gathered row):
  [k_b0 (128 bf16) | v_b0 | k_b1 | v_b1]
"""

import os
import sys

sys.path.insert(0, "/opt/trn_rl_repo")

from contextlib import ExitStack

import numpy as np

import concourse.bacc as bacc
import concourse.bass as bass
import concourse.tile as tile
from concourse import mybir

HEADS = 8
P = 128
NCALL = 4          # gather calls per tile (1024 idxs each)
GT = 8             # node-tiles per phase-1 group


class Cfg:
    def __init__(self, N=16384, K=32, C=128, n_cores=8, B=2):
        self.N, self.K, self.C, self.n_cores, self.B = N, K, C, n_cores, B
        self.N_own = N // n_cores
        self.n_all_tiles = N // P
        self.n_own_tiles = self.N_own // P
        self.d = C // HEADS


def _ap(base, dims):
    return bass.AP(tensor=base.tensor, offset=base.offset,
                   ap=[base.ap[0]] + [list(x) for x in dims])


def _off(base, elems):
    return bass.AP(tensor=base.tensor, offset=base.offset + elems,
                   ap=base.ap)


def build_nc(cfg: Cfg):
    N, K, C, B = cfg.N, cfg.K, cfg.C, cfg.B
    H3 = 3 * C
    R = 2 * B * C              # packed row elems (k|v per batch): 512
    d = cfg.d
    f32, bf16, i16 = mybir.dt.float32, mybir.dt.bfloat16, mybir.dt.int16
    T_all, T_own = cfg.n_all_tiles, cfg.n_own_tiles
    NG = T_all // GT           # phase-1 groups per batch (16)
    ni_call = K * P // NCALL   # 1024
    k_call = K // NCALL        # 8
    cols_call = ni_call // 16  # 64
    idx_cols = NCALL * cols_call
    M = K * B * C              # merged free size (8192)

    nc = bacc.Bacc("TRN2", target_bir_lowering=False, debug=False,
                   num_swdge_queues=4)

    xt = nc.dram_tensor("xt", [C, B * N], bf16, kind="ExternalInput")
    wqkv = nc.dram_tensor("wqkv", [C, H3], bf16, kind="ExternalInput")
    brow = nc.dram_tensor("brow", [C, H3], f32, kind="ExternalInput")
    idxw = nc.dram_tensor("idxw", [P, T_own * idx_cols], i16,
                          kind="ExternalInput")
    outp = nc.dram_tensor("out", [B * cfg.N_own, C], f32,
                          kind="ExternalOutput")

    with tile.TileContext(nc) as tc, ExitStack() as ctx:
        const = ctx.enter_context(tc.tile_pool(name="const", bufs=1))
        xload = ctx.enter_context(tc.tile_pool(name="xload", bufs=3))
        psum = ctx.enter_context(tc.tile_pool(name="psum", bufs=2, space="PSUM"))
        kvsb = ctx.enter_context(tc.tile_pool(name="kvsb", bufs=2))
        dram = ctx.enter_context(tc.tile_pool(name="dram", bufs=1, space="DRAM"))
        gath = ctx.enter_context(tc.tile_pool(name="gath", bufs=2))
        work = ctx.enter_context(tc.tile_pool(name="work", bufs=1))
        small = ctx.enter_context(tc.tile_pool(name="small", bufs=1))

        # --- constants ---
        wqkv_sb = const.tile([C, H3], bf16)
        nc.sync.dma_start(out=wqkv_sb[:], in_=wqkv[:, :])
        bqrep_sb = const.tile([P, C], f32)   # bq replicated across partitions
        q0 = brow[0:1, 0:C]
        nc.gpsimd.dma_start(
            out=bqrep_sb[:],
            in_=bass.AP(tensor=q0.tensor, offset=q0.offset, ap=[[0, P], [1, C]]))
        bvrep2_sb = const.tile([P, 2 * C], f32)  # bv twice: [(b, hd)]
        b0 = brow[0:1, 2 * C : 3 * C]
        nc.gpsimd.dma_start(
            out=bvrep2_sb[:],
            in_=bass.AP(tensor=b0.tensor, offset=b0.offset,
                        ap=[[0, P], [0, 2], [1, C]]))
        idx_sb = const.tile([P, T_own * idx_cols], i16)
        nc.sync.dma_start(out=idx_sb[:], in_=idxw[:, :])
        q_all = const.tile([P, T_own * B * C], bf16)   # [t][b][hd]

        kv_dram = dram.tile([N, R], bf16)

        # --- phase 1: k|v projections, all nodes, both batches ---
        # bk cancels in the softmax (constant over the neighbor axis) and
        # bv is added exactly at the end (sum_k attn == 1), so k|v rows are
        # written WITHOUT bias; only q gets its bias (DVE add below).
        for b in range(B):
            for g in range(NG):
                n0 = b * N + g * GT * P
                xt_g = xload.tile([C, GT * P], bf16, tag="xt")
                nc.sync.dma_start(out=xt_g[:], in_=xt[:, n0 : n0 + GT * P])
                pt = psum.tile([P, GT * 2 * C], f32, tag="pp")   # 4 banks
                for c in range(GT):
                    nc.tensor.matmul(out=pt[:, c * 2 * C : (c + 1) * 2 * C],
                                     lhsT=xt_g[:, c * P : (c + 1) * P],
                                     rhs=wqkv_sb[:, C:H3],
                                     start=True, stop=True)
                kv_t = kvsb.tile([P, GT * 2 * C], bf16, tag="kv")
                nc.scalar.activation(out=kv_t[:], in_=pt[:],
                                     func=mybir.ActivationFunctionType.Copy)
                # rows g*GT*P .. +GT*P, cols [2bC, 2bC+256) of kv_dram
                kvd = kv_dram[:]
                nc.sync.dma_start(
                    out=bass.AP(tensor=kvd.tensor,
                                offset=kvd.offset + g * GT * P * R + 2 * b * C,
                                ap=[[R, P], [P * R, GT], [1, 2 * C]]),
                    in_=kv_t[:])

        # q projections for own tiles (first T_own tiles of each batch)
        for b in range(B):
            for g in range(T_own // GT):        # 2 groups of 8 tiles
                n0 = b * N + g * GT * P
                xt_g = xload.tile([C, GT * P], bf16, tag="xt")
                nc.sync.dma_start(out=xt_g[:], in_=xt[:, n0 : n0 + GT * P])
                pq = psum.tile([P, GT * 2 * C], f32, tag="pp")
                for c in range(GT):
                    nc.tensor.matmul(out=pq[:, c * C : (c + 1) * C],
                                     lhsT=xt_g[:, c * P : (c + 1) * P],
                                     rhs=wqkv_sb[:, 0:C],
                                     start=True, stop=True)
                qa = q_all[:]
                nc.vector.tensor_tensor(
                    out=bass.AP(tensor=qa.tensor,
                                offset=qa.offset + (g * GT * B + b) * C,
                                ap=[qa.ap[0], [B * C, GT], [1, C]]),
                    in0=pq[:, 0 : GT * C],
                    in1=_ap(bqrep_sb[:], [[0, GT], [1, C]]),
                    op=mybir.AluOpType.add)

        # --- phase 2: gather + attention, software-pipelined ---
        # iter t: gather(t), prod(t), scores(t), exp(t) [ACT]
        #         then finish tile t-1: z, rz, prodv, tree, scale, store.
        prev = None
        for t in range(T_own + 1):
            if t < T_own:
                kvg = gath.tile([P, K, R], bf16, tag="kvg")
                for i in range(NCALL):
                    c0 = t * idx_cols + i * cols_call
                    nc.gpsimd.dma_gather(
                        out_ap=kvg[:, i * k_call : (i + 1) * k_call, :],
                        in_ap=kv_dram[:],
                        idxs_ap=idx_sb[:, c0 : c0 + cols_call],
                        num_idxs=ni_call,
                        num_idxs_reg=ni_call,
                        elem_size=R,
                        queue_num=i,
                    )

                qt = q_all[:, t * B * C : (t + 1) * B * C]     # [(b, hd)]

                # prod[(k,b,hd)] = kg * q   (bf16 2x)
                prod = work.tile([P, M], bf16, tag="prod")
                nc.vector.tensor_tensor(
                    out=_ap(prod[:], [[B * C, K], [C, B], [1, C]]),
                    in0=_ap(kvg[:, 0, 0:C], [[R, K], [2 * C, B], [1, C]]),
                    in1=_ap(qt, [[0, K], [C, B], [1, C]]),
                    op=mybir.AluOpType.mult)

                # scores[(b,h,k)] = sum_d prod  (grouped reduce over d;
                # out AP transposes group order k-major -> (b,h)-major so z
                # can reduce over a contiguous k run)
                scores = small.tile([P, K * B * HEADS], f32, tag="scores",
                                    bufs=2)
                nc.vector.tensor_reduce(
                    out=_ap(scores[:], [[1, K], [K, B * HEADS]]),
                    in_=_ap(prod[:], [[B * C, K], [d, B * HEADS], [1, d]]),
                    axis=mybir.AxisListType.X, op=mybir.AluOpType.add)

                # expx[(k,b,hd)] = exp(scores/4) expanded over d (ScalarE)
                expx = work.tile([P, M], bf16, tag="expx", bufs=2)
                for b in range(B):
                    nc.scalar.activation(
                        out=_ap(_off(expx[:], b * C),
                                [[B * C, K], [d, HEADS], [1, d]]),
                        in_=_ap(_off(scores[:], b * HEADS * K),
                                [[1, K], [K, HEADS], [0, d]]),
                        func=mybir.ActivationFunctionType.Exp,
                        scale=1.0 / float(np.sqrt(d)))
                # compact exp in (b,h,k) layout (ScalarE) for the cheap
                # contiguous z reduce
                cexp = small.tile([P, K * B * HEADS], bf16, tag="cexp",
                                  bufs=2)
                nc.scalar.activation(
                    out=cexp[:], in_=scores[:],
                    func=mybir.ActivationFunctionType.Exp,
                    scale=1.0 / float(np.sqrt(d)))
            else:
                kvg = expx = cexp = None

            if prev is not None:
                pkvg, pexpx, pcexp, pt_idx = prev

                # z[(b,h)] = sum_k cexp[b,h,k]  (contiguous k runs)
                z = small.tile([P, B * HEADS], f32, tag="z")
                nc.vector.tensor_reduce(
                    out=z[:],
                    in_=_ap(pcexp[:], [[K, B * HEADS], [1, K]]),
                    axis=mybir.AxisListType.X, op=mybir.AluOpType.add)
                rz = small.tile([P, B * HEADS], f32, tag="rz")
                nc.vector.reciprocal(rz[:], z[:])

                # prodv = expx * vg  (both dense, bf16 2x)
                prodv = work.tile([P, M], bf16, tag="prodv")
                nc.vector.tensor_tensor(
                    out=_ap(prodv[:], [[B * C, K], [C, B], [1, C]]),
                    in0=_ap(pkvg[:, 0, C : 2 * C], [[R, K], [2 * C, B], [1, C]]),
                    in1=_ap(pexpx[:], [[B * C, K], [C, B], [1, C]]),
                    op=mybir.AluOpType.mult)

                # acc[(b,hd)] = sum_k prodv : 5-level pairwise tree over k
                at1 = work.tile([P, M // 2], bf16, tag="at1")
                nc.vector.tensor_tensor(
                    out=at1[:], in0=prodv[:, 0 : M // 2],
                    in1=prodv[:, M // 2 : M], op=mybir.AluOpType.add)
                at2 = small.tile([P, M // 4], bf16, tag="at2")
                nc.vector.tensor_tensor(
                    out=at2[:], in0=at1[:, 0 : M // 4],
                    in1=at1[:, M // 4 : M // 2], op=mybir.AluOpType.add)
                at3 = small.tile([P, M // 8], bf16, tag="at3")
                nc.vector.tensor_tensor(
                    out=at3[:], in0=at2[:, 0 : M // 8],
                    in1=at2[:, M // 8 : M // 4], op=mybir.AluOpType.add)
                at4 = small.tile([P, M // 16], bf16, tag="at4")
                nc.vector.tensor_tensor(
                    out=at4[:], in0=at3[:, 0 : M // 16],
                    in1=at3[:, M // 16 : M // 8], op=mybir.AluOpType.add)
                acc = small.tile([P, B * C], f32, tag="acc")
                nc.vector.tensor_tensor(
                    out=acc[:], in0=at4[:, 0 : B * C],
                    in1=at4[:, B * C : 2 * B * C], op=mybir.AluOpType.add)

                # out = acc * (1/z) + bv   (bv exact: sum_k attn == 1)
                sc = small.tile([P, B * C], f32, tag="sc")
                nc.vector.tensor_tensor(
                    out=_ap(sc[:], [[C, B], [d, HEADS], [1, d]]),
                    in0=_ap(acc[:], [[C, B], [d, HEADS], [1, d]]),
                    in1=_ap(rz[:], [[HEADS, B], [1, HEADS], [0, d]]),
                    op=mybir.AluOpType.mult)
                outt = small.tile([P, B * C], f32, tag="outt", bufs=2)
                nc.vector.tensor_tensor(
                    out=outt[:], in0=sc[:], in1=bvrep2_sb[:],
                    op=mybir.AluOpType.add)
                op_ = outp[:, :]
                nc.sync.dma_start(
                    out=bass.AP(tensor=op_.tensor,
                                offset=op_.offset + pt_idx * P * C,
                                ap=[[C, P], [cfg.N_own * C, B], [1, C]]),
                    in_=outt[:])

            prev = (kvg, expx, cexp, t) if t < T_own else None

    nc.compile()
    return nc


def make_in_maps(cfg: Cfg, x, Wq, bq, Wk, bk, Wv, bv, neighbor_index):
    N, K, C, B = cfg.N, cfg.K, cfg.C, cfg.B
    T_own, N_own = cfg.n_own_tiles, cfg.N_own

    x = np.asarray(x, np.float32)
    wqkv = np.ascontiguousarray(np.concatenate(
        [np.asarray(Wq, np.float32).T, np.asarray(Wk, np.float32).T,
         np.asarray(Wv, np.float32).T], axis=1))
    wqkv_bf = _to_bf16(wqkv)
    brow = np.zeros((C, 3 * C), np.float32)
    brow[0, :] = np.concatenate(
        [np.asarray(bq, np.float32), np.asarray(bk, np.float32),
         np.asarray(bv, np.float32)])
    nbr = np.asarray(neighbor_index, np.int64)
    xtb = np.ascontiguousarray(x.transpose(0, 2, 1))   # [B, C, N]

    in_maps = []
    for c in range(cfg.n_cores):
        own = np.arange(c * N_own, (c + 1) * N_own)
        rest = np.concatenate(
            [np.arange(0, c * N_own), np.arange((c + 1) * N_own, N)])
        perm = np.concatenate([own, rest])
        inv = np.empty(N, np.int64)
        inv[perm] = np.arange(N)

        xt_c = np.ascontiguousarray(
            xtb[:, :, perm].transpose(1, 0, 2).reshape(C, B * N))

        nb = inv[nbr[own]]                                  # [N_own, K]
        vals = nb.reshape(T_own, P, K).transpose(0, 2, 1)   # [T, k, nl]
        vals = vals.reshape(T_own, NCALL, (K // NCALL) * P)
        a = vals.reshape(T_own, NCALL, (K // NCALL) * P // 16, 16)
        a = a.transpose(3, 0, 1, 2)                          # [16, T, NCALL, S]
        rep = np.tile(a, (8, 1, 1, 1))                       # [128, ...]
        idxw = np.ascontiguousarray(
            rep.reshape(P, T_own * (K * P // 16)).astype(np.int16))

        in_maps.append({
            "xt": _to_bf16(xt_c), "wqkv": wqkv_bf, "brow": brow,
            "idxw": idxw,
        })
    return in_maps


def _to_bf16(a: np.ndarray) -> np.ndarray:
    import ml_dtypes
    return np.ascontiguousarray(a.astype(ml_dtypes.bfloat16))


_CACHE = {}


def _get_nc(cfg: Cfg):
    key = (cfg.N, cfg.K, cfg.C, cfg.n_cores, cfg.B)
    if key not in _CACHE:
        _CACHE[key] = build_nc(cfg)
    return _CACHE[key]


def kernel(x, Wq, bq, Wk, bk, Wv, bv, neighbor_index, _trace=False):
    from concourse.bass_utils import run_bass_kernel_spmd

    x = np.asarray(x)
    B, N, C = x.shape
    K = np.asarray(neighbor_index).shape[1]
    cfg = Cfg(N=N, K=K, C=C, n_cores=8, B=B)
    nc = _get_nc(cfg)
    in_maps = make_in_maps(cfg, x, Wq, bq, Wk, bk, Wv, bv, neighbor_index)
    res = run_bass_kernel_spmd(nc, in_maps, core_ids=list(range(cfg.n_cores)),
                               trace=_trace)
    out = np.empty((B, N, C), np.float32)
    for c in range(cfg.n_cores):
        o = res.results[c]["out"].reshape(B, cfg.N_own, C)
        out[:, c * cfg.N_own : (c + 1) * cfg.N_own, :] = o
    if _trace:
        kernel.last_results = res
    return out
